# revision 1
# baseline (speedup 1.0000x reference)
"""Cross-attention Trainium2 Bass kernel.

Sharding: B*T rows of `tokens` split across 8 cores (each core takes one
batch's half: core c -> batch c//2, t-rows [ (c%2)*2048, +2048 )).  Each core
computes its full slice of the output independently (K/V projections for its
batch are recomputed on the 2 cores sharing that batch); host gather is a
pure concatenation.

Per-core pipeline (all matmuls bf16 with fp32 PSUM accumulation):
  tokens/context -> (SWDGE cast DMA) bf16 DRAM -> (HWDGE transpose DMA)
  tokens^T / context^T in SBUF
  K^T = Wk^T ctx^T, V = ctx Wv (with a ones-column appended per head),
  Q^T = Wq^T tok^T
  per head pair (row-packed K=64 matmuls): scores^T = K^T.T Q^T -> exp on ACT
  -> P^T (bf16) -> ctx_aug^T = V_aug^T P^T (row 64 = softmax denominator)
  -> normalize via DVE reciprocal + gpsimd partition_broadcast + DVE mul
  out = ctx_norm Wo + bo  (lhsT = ctx_norm^T chunks, K=128)
"""

import math
from contextlib import ExitStack

import numpy as np

import concourse.bass as bass
import concourse.mybir as mybir
import concourse.tile as tile
from concourse import bacc
from concourse.bass_utils import run_bass_kernel_spmd

B, T, S = 4, 4096, 1024
HID, CTX, EMB, H = 1024, 768, 1024, 16
D = EMB // H  # 64
NCORES = 8
TC = (B * T) // NCORES  # 2048 rows of tokens per core
SCALE = 1.0 / math.sqrt(D)

F32 = mybir.dt.float32
BF16 = mybir.dt.bfloat16
EXP = mybir.ActivationFunctionType.Exp


def build(debug=False, repeat=1):
    nc = bacc.Bacc("TRN2", target_bir_lowering=False, debug=False,
                   num_devices=NCORES)
    tokens = nc.dram_tensor("tokens", [TC, HID], F32, kind="ExternalInput")
    ctx_in = nc.dram_tensor("context", [S, CTX], F32, kind="ExternalInput")
    wq = nc.dram_tensor("Wq", [HID, EMB], F32, kind="ExternalInput")
    wk = nc.dram_tensor("Wk", [CTX, EMB], F32, kind="ExternalInput")
    wv = nc.dram_tensor("Wv", [CTX, EMB], F32, kind="ExternalInput")
    wo = nc.dram_tensor("Wo", [EMB, HID], F32, kind="ExternalInput")
    bo = nc.dram_tensor("bo", [HID], F32, kind="ExternalInput")
    out = nc.dram_tensor("out", [TC, HID], F32, kind="ExternalOutput")
    dbg = {}
    if debug:
        dbg["qt"] = nc.dram_tensor("dbg_qt", [EMB, TC], F32, kind="ExternalOutput")
        dbg["kt"] = nc.dram_tensor("dbg_kt", [EMB, S], F32, kind="ExternalOutput")
        dbg["v"] = nc.dram_tensor("dbg_v", [S, H, D + 1], F32, kind="ExternalOutput")
        dbg["ctxn"] = nc.dram_tensor("dbg_ctxn", [EMB, TC], F32, kind="ExternalOutput")

    with tile.TileContext(nc) as tc, ExitStack() as outer:
        wpool = outer.enter_context(tc.tile_pool(name="weights", bufs=1))
        qkv = outer.enter_context(tc.tile_pool(name="qkv", bufs=1))
        psum = outer.enter_context(tc.tile_pool(name="psum", bufs=1, space="PSUM"))

        def psum_big():
            # shared 2-bank slots used by projections, scores and out-proj
            return psum.tile([128, 1024], F32, name="big", tag="big", bufs=2)

        for _it in range(repeat):
            wk_sb = [wpool.tile([128, EMB], BF16, name=f"wk{i}", tag=f"wk{i}") for i in range(6)]
            wv_sb = [wpool.tile([128, EMB], BF16, name=f"wv{i}", tag=f"wv{i}") for i in range(6)]
            wq_sb = [wpool.tile([128, EMB], BF16, name=f"wq{i}", tag=f"wq{i}") for i in range(8)]
            kt = [qkv.tile([128, S], BF16, name=f"kt{i}", tag=f"kt{i}") for i in range(8)]
            v_sb = [qkv.tile([128, H, D + 1], BF16, name=f"v{i}", tag=f"v{i}") for i in range(8)]
            qt = [qkv.tile([128, TC], BF16, name=f"qt{i}", tag=f"qt{i}") for i in range(8)]

            with ExitStack() as proj:
                dram = proj.enter_context(tc.tile_pool(name="dram", bufs=1, space="DRAM"))
                tmp = proj.enter_context(tc.tile_pool(name="tmp", bufs=1))

                # Input preprocessing on HWDGE + ACT only (keeps the single
                # SWDGE queue free for weight-cast DMAs): fp32 load -> ACT
                # cast -> bf16 DRAM store -> HWDGE transpose load.
                fstage = proj.enter_context(tc.tile_pool(name="fstage", bufs=3))
                gstage = proj.enter_context(tc.tile_pool(name="gstage", bufs=3))
                ctx16 = dram.tile([S, CTX], BF16, name="ctx16", tag="ctx16")
                tok16 = dram.tile([TC, HID], BF16, name="tok16", tag="tok16")
                ctxT = [tmp.tile([128, S], BF16, name=f"ctxT{i}", tag=f"ctxT{i}") for i in range(6)]
                tokT = [tmp.tile([128, TC], BF16, name=f"tokT{i}", tag=f"tokT{i}") for i in range(8)]
                for r in range(8):
                    f = fstage.tile([128, CTX], F32, name="cf", tag="cf")
                    g = gstage.tile([128, CTX], BF16, name="cg", tag="cg")
                    nc.sync.dma_start(out=f[:, :], in_=ctx_in.ap()[r * 128:(r + 1) * 128, :])
                    nc.scalar.copy(g[:, :], f[:, :])
                    nc.scalar.dma_start(out=ctx16[r * 128:(r + 1) * 128, :], in_=g[:, :])
                for i in range(6):
                    nc.sync.dma_start(out=ctxT[i][:, :],
                                      in_=ctx16[:, i * 128:(i + 1) * 128], transpose=True)
                    nc.gpsimd.dma_start(out=wk_sb[i][:, :], in_=wk.ap()[i * 128:(i + 1) * 128, :])
                    nc.gpsimd.dma_start(out=wv_sb[i][:, :], in_=wv.ap()[i * 128:(i + 1) * 128, :])
                for r in range(16):
                    f = fstage.tile([128, HID], F32, name="tf", tag="tf")
                    g = gstage.tile([128, HID], BF16, name="tg", tag="tg")
                    nc.sync.dma_start(out=f[:, :], in_=tokens.ap()[r * 128:(r + 1) * 128, :])
                    nc.scalar.copy(g[:, :], f[:, :])
                    nc.scalar.dma_start(out=tok16[r * 128:(r + 1) * 128, :], in_=g[:, :])
                for i in range(8):
                    nc.sync.dma_start(out=tokT[i][:, :],
                                      in_=tok16[:, i * 128:(i + 1) * 128], transpose=True)
                    nc.gpsimd.dma_start(out=wq_sb[i][:, :], in_=wq.ap()[i * 128:(i + 1) * 128, :])

                # K^T[e, s] accumulated over 6 context chunks
                for e in range(8):
                    ps = psum_big()
                    for sh in range(2):
                        for c in range(6):
                            nc.tensor.matmul(ps[:, sh * 512:(sh + 1) * 512],
                                             lhsT=wk_sb[c][:, e * 128:(e + 1) * 128],
                                             rhs=ctxT[c][:, sh * 512:(sh + 1) * 512],
                                             start=(c == 0), stop=(c == 5))
                    nc.vector.tensor_copy(kt[e][:, :], ps[:, :])
                # V[s, d] (+ ones column per head)
                for sc in range(8):
                    ps = psum_big()
                    for dh in range(2):
                        for c in range(6):
                            nc.tensor.matmul(ps[:, dh * 512:(dh + 1) * 512],
                                             lhsT=ctxT[c][:, sc * 128:(sc + 1) * 128],
                                             rhs=wv_sb[c][:, dh * 512:(dh + 1) * 512],
                                             start=(c == 0), stop=(c == 5))
                    nc.vector.tensor_copy(
                        v_sb[sc][:, :, 0:D],
                        ps[:, :].rearrange("p (h d) -> p h d", d=D))
                    nc.vector.memset(v_sb[sc][:, :, D:D + 1], 1.0)
                # Q^T[e, t]
                for e in range(8):
                    for tq in range(2):
                        ps = psum_big()
                        for hc in range(8):
                            for sh in range(2):
                                nc.tensor.matmul(
                                    ps[:, sh * 512:(sh + 1) * 512],
                                    lhsT=wq_sb[hc][:, e * 128:(e + 1) * 128],
                                    rhs=tokT[hc][:, tq * 1024 + sh * 512:tq * 1024 + (sh + 1) * 512],
                                    start=(hc == 0), stop=(hc == 7))
                        nc.vector.tensor_copy(qt[e][:, tq * 1024:(tq + 1) * 1024], ps[:, :])

            if debug:
                for e in range(8):
                    nc.gpsimd.dma_start(out=dbg["qt"].ap()[e * 128:(e + 1) * 128, :], in_=qt[e][:, :])
                    nc.gpsimd.dma_start(out=dbg["kt"].ap()[e * 128:(e + 1) * 128, :], in_=kt[e][:, :])
                for sc in range(8):
                    nc.gpsimd.dma_start(
                        out=dbg["v"].ap()[sc * 128:(sc + 1) * 128, :, :], in_=v_sb[sc][:, :, :])

            # load Wo/bo while attention runs
            wo_sb = [wpool.tile([128, HID], BF16, name=f"wo{i}", tag=f"wo{i}") for i in range(8)]
            bo_b = wpool.tile([128, HID], F32, name="bo", tag="bo")
            for i in range(8):
                nc.gpsimd.dma_start(out=wo_sb[i][:, :], in_=wo.ap()[i * 128:(i + 1) * 128, :])
            b_ap = bo.ap()
            nc.gpsimd.dma_start(
                out=bo_b[:, :],
                in_=bass.AP(tensor=b_ap.tensor, offset=b_ap.offset,
                            ap=[[0, 128]] + list(b_ap.ap)))

            # ---------------- attention + per-half output projection
            with ExitStack() as attn:
                ctxn_pool = attn.enter_context(tc.tile_pool(name="ctxn", bufs=1))
                apool = attn.enter_context(tc.tile_pool(name="apool", bufs=4))
                npool = attn.enter_context(tc.tile_pool(name="npool", bufs=2))
                opool = attn.enter_context(tc.tile_pool(name="opool", bufs=3))

                for th in range(2):  # t half of 1024
                    t0 = th * 1024
                    ctxn = [ctxn_pool.tile([128, 1024], BF16, name=f"ctxn{i}_{th}",
                                           tag=f"ctxn{i}") for i in range(8)]
                    for p in range(8):  # head pair: 2p (rows 0-63), 2p+1 (64-127)
                        hA, hB = 2 * p, 2 * p + 1
                        cA = [psum.tile([65, 512], F32, name="ctx", tag="ctx", bufs=4)
                              for _ in range(2)]
                        cB = [psum.tile([65, 512], F32, name="ctx", tag="ctx", bufs=4)
                              for _ in range(2)]
                        for sc in range(8):  # s chunks of 128
                            sA = psum_big()
                            sB = psum_big()
                            for tt in range(2):
                                ts0 = t0 + tt * 512
                                nc.tensor.matmul(sA[:, tt * 512:(tt + 1) * 512],
                                                 lhsT=kt[p][0:64, sc * 128:(sc + 1) * 128],
                                                 rhs=qt[p][0:64, ts0:ts0 + 512],
                                                 start=True, stop=True)
                                nc.tensor.matmul(sB[:, tt * 512:(tt + 1) * 512],
                                                 lhsT=kt[p][64:128, sc * 128:(sc + 1) * 128],
                                                 rhs=qt[p][64:128, ts0:ts0 + 512],
                                                 start=True, stop=True)
                            pA = apool.tile([128, 1024], BF16, name="p", tag="p")
                            pB = apool.tile([128, 1024], BF16, name="p", tag="p")
                            nc.scalar.activation(pA[:, :], sA[:, :], EXP, scale=SCALE)
                            nc.scalar.activation(pB[:, :], sB[:, :], EXP, scale=SCALE)
                            for tt in range(2):
                                nc.tensor.matmul(cA[tt][:, :], lhsT=v_sb[sc][:, hA, :],
                                                 rhs=pA[:, tt * 512:(tt + 1) * 512],
                                                 start=(sc == 0), stop=(sc == 7))
                                nc.tensor.matmul(cB[tt][:, :], lhsT=v_sb[sc][:, hB, :],
                                                 rhs=pB[:, tt * 512:(tt + 1) * 512],
                                                 start=(sc == 0), stop=(sc == 7))
                        # Evacuate ctx PSUM banks to SBUF right away (frees the
                        # banks), then normalize out of SBUF at 1024 grain.
                        stA = npool.tile([65, 1024], F32, name="stA", tag="stA")
                        stB = npool.tile([65, 1024], F32, name="stB", tag="stB")
                        for tt in range(2):
                            nc.vector.tensor_copy(stA[:, tt * 512:(tt + 1) * 512], cA[tt][:, :])
                            nc.vector.tensor_copy(stB[:, tt * 512:(tt + 1) * 512], cB[tt][:, :])
                        for st, base in ((stA, 0), (stB, 64)):
                            rc = npool.tile([65, 1024], F32, name="recip", tag="recip", bufs=1)
                            rc0 = npool.tile([1, 1024], F32, name="recip0", tag="recip0")
                            rep = npool.tile([64, 1024], F32, name="rep", tag="rep")
                            nc.vector.reciprocal(rc[64:65, :], st[64:65, :])
                            # partition shift 64 -> 0 via DMA, then broadcast
                            nc.gpsimd.dma_start(out=rc0[0:1, :], in_=rc[64:65, :])
                            nc.gpsimd.partition_broadcast(rep[:, :], rc0[0:1, :])
                            if base == 0:
                                nc.vector.tensor_mul(ctxn[p][0:64, :],
                                                     st[0:64, :], rep[:, :])
                            else:
                                # in-place normalize, then cast+partition-shift DMA
                                nc.vector.tensor_mul(st[0:64, :], st[0:64, :], rep[:, :])
                                nc.gpsimd.dma_start(out=ctxn[p][64:128, :],
                                                    in_=st[0:64, :])

                    if debug:
                        for e in range(8):
                            nc.gpsimd.dma_start(
                                out=dbg["ctxn"].ap()[e * 128:(e + 1) * 128, t0:t0 + 1024],
                                in_=ctxn[e][:, :])

                    # output projection for this t half
                    for tck in range(8):
                        ps = psum_big()
                        for nh in range(2):
                            ot = opool.tile([128, 512], F32, name="o", tag="o")
                            for p in range(8):
                                nc.tensor.matmul(ps[:, nh * 512:(nh + 1) * 512],
                                                 lhsT=ctxn[p][:, tck * 128:(tck + 1) * 128],
                                                 rhs=wo_sb[p][:, nh * 512:(nh + 1) * 512],
                                                 start=(p == 0), stop=(p == 7))
                            nc.vector.tensor_add(ot[:, :], ps[:, nh * 512:(nh + 1) * 512],
                                                 bo_b[:, nh * 512:(nh + 1) * 512])
                            nc.sync.dma_start(
                                out=out.ap()[t0 + tck * 128:t0 + (tck + 1) * 128,
                                             nh * 512:(nh + 1) * 512],
                                in_=ot[:, :])

    nc.compile()
    return nc


_CACHE = {}


def _get_nc(debug=False, repeat=1):
    key = (debug, repeat)
    if key not in _CACHE:
        _CACHE[key] = build(debug, repeat)
    return _CACHE[key]


def make_in_maps(tokens, context, Wq, Wk, Wv, Wo, bo):
    tokens = np.ascontiguousarray(np.asarray(tokens, dtype=np.float32))
    context = np.ascontiguousarray(np.asarray(context, dtype=np.float32))
    shared = {
        "Wq": np.ascontiguousarray(np.asarray(Wq, dtype=np.float32)),
        "Wk": np.ascontiguousarray(np.asarray(Wk, dtype=np.float32)),
        "Wv": np.ascontiguousarray(np.asarray(Wv, dtype=np.float32)),
        "Wo": np.ascontiguousarray(np.asarray(Wo, dtype=np.float32)),
        "bo": np.ascontiguousarray(np.asarray(bo, dtype=np.float32)),
    }
    in_maps = []
    for c in range(NCORES):
        b, t0 = c // 2, (c % 2) * TC
        in_maps.append({
            "tokens": np.ascontiguousarray(tokens[b, t0:t0 + TC, :]),
            "context": np.ascontiguousarray(context[b]),
            **shared,
        })
    return in_maps


def kernel(tokens, context, Wq, Wk, Wv, Wo, bo, _debug=False):
    nc = _get_nc(_debug)
    in_maps = make_in_maps(tokens, context, Wq, Wk, Wv, Wo, bo)
    res = run_bass_kernel_spmd(nc, in_maps, core_ids=list(range(NCORES)))
    out = np.empty((B, T, HID), dtype=np.float32)
    for c in range(NCORES):
        b, t0 = c // 2, (c % 2) * TC
        out[b, t0:t0 + TC, :] = res.results[c]["out"]
    if _debug:
        return out, res
    return out

